# revision 16
# baseline (speedup 1.0000x reference)
"""Trainium2 Bass kernel for nn_AutoencoderHybrid (12-qubit QAE hybrid).

Math: the circuit measures Z on wires 0..3 only. The CNOT chain (i -> i+1)
propagates information forward only, so in the Heisenberg picture each
observable Z_w pulled back through the 2-layer circuit is supported on wires
0..w+1 (at most wires 0..4). With the product input state
|psi_b> = kron_j [cos(x_j/2), -i sin(x_j/2)], the diagonal phase factors
D = diag((-i)^popcount) fold into the observable, leaving a REAL quadratic
form on a real product vector. Moreover S_w = Stil_w (x) I_{2^(3-w)} — the
quadform for observable w contracts over only d_w = 2^(w+2) dims:

    latent_w(b) = v_w^T Stil_w v_w,  v_w = kron_{j<=w+1} [cos(x_j/2), sin(x_j/2)]

The prefix vectors A(4) = v_0, B(8) = v_1, D(16) = v_2, R(32) = v_3 all fall
out of the same kron tree. Device computes per batch row the concat vector
vcat = [R|D|B|A] (60 slots, 64-padded) per group, one PE matmul against the
block-diagonal prefix matrix (240 cols vs 512 for the dense form), an
elementwise multiply + ragged grouped reductions, then the tiny decoder MLP
in transposed space (b1 folded into the ACT relu bias, b2 into the final
PSUM->SBUF add).

S_w / MLP weights are tiny (depend only on q_params etc.) and are prepared on
the host; all batch-dim work (B = 8192) runs on 8 NeuronCores, data parallel,
1024 rows per core laid out as b = 8p + c (p partition, c free-dim group).

Scheduling: the x DMA instruction is hoisted into the entry block BEFORE the
all-engine start barrier (saves ~700ns of DMA pipeline latency); constants
arrive as two packed DMAs right behind it on the SP queue; the two output
halves are DMA'd out independently (SP + ACT queues) as soon as each is
written.
"""
import math
import numpy as np

N5 = 5
NLAYERS = 2
LATENT = 4
B = 8192
NCORES = 8
BLOC = B // NCORES  # 1024

# slot layout per group inside vcat (64-wide): [R:0..32|D:32..48|B:48..56|A:56..60]
_SLOT_OFF = {3: 0, 2: 32, 1: 48, 0: 56}
_SLOT_DIM = {3: 32, 2: 16, 1: 8, 0: 4}

# ----------------------------------------------------------------------------
# Host-side constant construction (pure numpy)
# ----------------------------------------------------------------------------


def _rot(phi, theta, omega):
    c, s = np.cos(theta / 2), np.sin(theta / 2)
    ep = np.exp(-0.5j * (phi + omega))
    em = np.exp(-0.5j * (phi - omega))
    return np.array([[ep * c, -np.conj(em) * s], [em * s, np.conj(ep) * c]],
                    dtype=np.complex128)


def _build_S(q_params):
    """(4, 32, 32) real symmetric: latent_w = r^T S_w r (unit-norm slots)."""
    qp = np.asarray(q_params, np.float64)
    dim = 2 ** N5
    eye2 = np.eye(2)

    def kron_at(U, wire):
        M = np.array([[1.0]])
        for j in range(N5):
            M = np.kron(M, U if j == wire else eye2)
        return M

    def cnot_mat(c, t):
        M = np.zeros((dim, dim))
        for z in range(dim):
            bits = [(z >> (N5 - 1 - j)) & 1 for j in range(N5)]
            if bits[c] == 1:
                bits[t] ^= 1
            z2 = 0
            for b in bits:
                z2 = (z2 << 1) | b
            M[z2, z] = 1.0
        return M

    V = np.eye(dim, dtype=np.complex128)
    for l in range(NLAYERS):
        for i in range(N5):
            V = kron_at(_rot(*qp[l, i]), i) @ V
        for i in range(N5 - 1):
            V = cnot_mat(i, i + 1) @ V

    pc = np.array([bin(z).count("1") for z in range(dim)])
    D = np.diag((-1j) ** pc)
    VD = V @ D
    Ss = []
    for w in range(LATENT):
        zdiag = np.array([1.0 if ((z >> (N5 - 1 - w)) & 1) == 0 else -1.0
                          for z in range(dim)])
        O = VD.conj().T @ (zdiag[:, None] * VD)
        Ss.append(np.real(O))
    return np.stack(Ss)


def _host_consts(q_params, W1, b1, W2, b2):
    S = _build_S(q_params)  # (4, 32, 32), unscaled
    # prefix block-diagonal quadform matrix, device slots carry cos/2, sin/2
    # so fold 4^(w+2) per observable
    M1 = np.zeros((64, 60))
    for w in range(4):
        d = _SLOT_DIM[w]
        o = _SLOT_OFF[w]
        tail = 32 // d
        M1[o:o + d, o:o + d] = S[w][::tail, ::tail] * (4.0 ** (w + 2))
    mproj = np.zeros((128, 120), np.float32)
    mproj[0:64, 0:60] = M1
    mproj[64:128, 60:120] = M1

    W1 = np.asarray(W1, np.float64)
    b1 = np.asarray(b1, np.float64)
    W2 = np.asarray(W2, np.float64)
    b2 = np.asarray(b2, np.float64)

    # lat layout [p, 4*ws + g], ws order [w3, w2, w1, w0]
    w1t = np.zeros((16, 128), np.float32)
    for ws in range(4):
        w = 3 - ws
        for g in range(4):
            w1t[4 * ws + g, 32 * g:32 * g + 32] = W1[:, w]
    w2blk = np.zeros((128, 48), np.float32)
    for g in range(4):
        w2blk[32 * g:32 * g + 32, 12 * g:12 * g + 12] = W2.T

    # PE consts: [mproj(0:120) | w2blk(120:168) | w1t(168:296 rows 0:16)]
    pecst = np.zeros((128, 296), np.float32)
    pecst[:, 0:120] = mproj
    pecst[:, 120:168] = w2blk
    pecst[0:16, 168:296] = w1t

    # vector consts: [b2rep(0:48) | b1T(48)]
    vcst = np.zeros((128, 52), np.float32)
    vcst[:, 0:48] = np.tile(b2, 4)[None, :]
    vcst[:, 48] = np.tile(b1, 4)
    return dict(pecst=pecst, vcst=vcst)


# ----------------------------------------------------------------------------
# Device kernel body (Bass/Tile)
# ----------------------------------------------------------------------------


def _build_body(ctx, tc, x, pecst, vcst, y):
    import concourse.bass as bass
    from concourse import mybir
    nc = tc.nc
    f32 = mybir.dt.float32
    f32r = mybir.dt.float32r
    AF = mybir.ActivationFunctionType
    AX = mybir.AxisListType
    ALU = mybir.AluOpType

    def fv(t, col, dims):
        """View of tile t at free-offset col with custom free dims."""
        return bass.AP(tensor=t.tensor, offset=t.offset + col,
                       ap=[list(t.ap[0])] + [list(d) for d in dims])

    consts = ctx.enter_context(tc.tile_pool(name="consts", bufs=1))
    sb = ctx.enter_context(tc.tile_pool(name="sb", bufs=1))
    sb2 = ctx.enter_context(tc.tile_pool(name="sb2", bufs=2))
    ps = ctx.enter_context(tc.tile_pool(name="ps", bufs=2, space="PSUM"))

    # ---- x load: hoisted before the entry barrier post-schedule (SP queue)
    x_s = sb.tile([128, 96], f32)
    xa = bass.AP(tensor=x.tensor, offset=0, ap=[[96, 128], [1, 96]])
    xdma = nc.sync.dma_start(x_s[:, :], xa)

    # ---- packed constants, also SP queue (issues right behind x)
    pe_s = consts.tile([128, 296], f32r)
    nc.sync.dma_start(pe_s[:, :], pecst.bitcast(f32r))
    v_s = consts.tile([128, 52], f32)
    nc.sync.dma_start(v_s[:, :], vcst)
    mproj_v = pe_s[:, 0:120]
    w2_v = pe_s[:, 120:168]
    w1t_v = pe_s[0:16, 168:296]
    b2_v = v_s[:, 0:48]
    b1_v = v_s[:, 48:49]

    bias_c = consts.tile([128, 1], f32)
    nc.vector.memset(bias_c[:, :], math.pi / 2)
    # ident built entirely on Pool (idle at start, no cross-engine hop)
    ones128 = consts.tile([128, 128], f32)
    nc.gpsimd.memset(ones128[:, :], 1.0)
    ident = consts.tile([128, 128], f32)
    nc.gpsimd.affine_select(out=ident[:, :], in_=ones128[:, :],
                            pattern=[[1, 128]],
                            compare_op=ALU.is_equal, fill=0.0,
                            base=0, channel_multiplier=-1)

    # ---- quarter angle: c4 = cos(x/4) FIRST (sq/cs_c only need c4), then s4
    # both halves in one op each: sc4[p, 10c + j] = s4, 10c + 5 + j = c4
    sc4 = sb.tile([128, 80], f32)
    xin = fv(x_s, 0, [[12, 8], [1, 5]])
    nc.scalar.activation(fv(sc4, 5, [[10, 8], [1, 5]]), xin, AF.Sin,
                         bias=bias_c[:, 0:1], scale=0.25)
    nc.scalar.activation(fv(sc4, 0, [[10, 8], [1, 5]]), xin, AF.Sin,
                         bias=0.0, scale=0.25)

    # ---- per-half slot assembly + prefix kron tree into vcat
    # cs[p, 10c + j] = cos(x_j/2)/2 = c4^2 - 1/2 ; 10c+5+j = sin(x_j/2)/2
    # vcat per group g (64-aligned): [R(32) | D(16) | B(8) | A(4) | pad]
    vch, csh = [], []
    for h in (0, 1):
        E = nc.vector if h == 0 else nc.gpsimd
        sqt = sb.tile([128, 20], f32, name=f"sq{h}")
        cst = sb.tile([128, 40], f32, name=f"cs{h}")
        # sq and cs_c depend only on c4 (first sin op); cs_s needs s4 too
        E.tensor_mul(fv(sqt, 0, [[5, 4], [1, 5]]),
                     fv(sc4, 40 * h + 5, [[10, 4], [1, 5]]),
                     fv(sc4, 40 * h + 5, [[10, 4], [1, 5]]))
        E.tensor_scalar_sub(fv(cst, 0, [[10, 4], [1, 5]]),
                            fv(sqt, 0, [[5, 4], [1, 5]]), 0.5)
        E.tensor_mul(fv(cst, 5, [[10, 4], [1, 5]]),
                     fv(sc4, 40 * h, [[10, 4], [1, 5]]),
                     fv(sc4, 40 * h + 5, [[10, 4], [1, 5]]))
        vc = sb.tile([128, 256], f32, name=f"vc{h}")
        # A[64g + 56 + 2 z0 + z1] = u0[z0] * u1[z1]
        E.tensor_mul(fv(vc, 56, [[64, 4], [2, 2], [1, 2]]),
                     fv(cst, 1, [[10, 4], [0, 2], [5, 2]]),
                     fv(cst, 0, [[10, 4], [5, 2], [0, 2]]))
        # B[64g + 48 + 2 a + z2] = A[a] * u2[z2]
        E.tensor_mul(fv(vc, 48, [[64, 4], [2, 4], [1, 2]]),
                     fv(vc, 56, [[64, 4], [1, 4], [0, 2]]),
                     fv(cst, 2, [[10, 4], [0, 4], [5, 2]]))
        # D[64g + 32 + 2 b + z3] = B[b] * u3[z3]
        E.tensor_mul(fv(vc, 32, [[64, 4], [2, 8], [1, 2]]),
                     fv(vc, 48, [[64, 4], [1, 8], [0, 2]]),
                     fv(cst, 3, [[10, 4], [0, 8], [5, 2]]))
        # R[64g + 2 d + z4] = D[d] * u4[z4]
        E.tensor_mul(fv(vc, 0, [[64, 4], [2, 16], [1, 2]]),
                     fv(vc, 32, [[64, 4], [1, 16], [0, 2]]),
                     fv(cst, 4, [[10, 4], [0, 16], [5, 2]]))
        vch.append(vc)
        csh.append(cst)

    # ---- per half: PE transposes -> DVE copies -> PE quadform matmuls
    # (PE in-order: keep h0's matmuls ahead of h1's transposes is WRONG --
    # emit per-half T,T,MM,MM so h0's matmuls don't wait on h1's kron)
    ybank = ps.tile([128, 480], f32, tag="Y", bufs=1)
    lat_all = sb.tile([128, 32], f32)
    Pm = {}
    for h in (0, 1):
        tps = []
        for c in (0, 1):
            tp = ps.tile([128, 128], f32, tag="tp", bufs=4)
            nc.tensor.transpose(tp[:, :], vch[h][:, 128 * c:128 * c + 128],
                                ident[:, :])
            tps.append(tp)
        vTs = []
        for c in (0, 1):
            vs = sb2.tile([128, 128], f32r, tag=f"vT{h}{c}", bufs=1)
            nc.vector.tensor_copy(vs[:, :], tps[c][:, :])
            vTs.append(vs)
        for c in (0, 1):
            o = 240 * h + 120 * c
            nc.tensor.matmul(ybank[:, o:o + 120], lhsT=vTs[c][:, :],
                             rhs=mproj_v, start=True, stop=True)
        # Pm = Y * vcat slots, then ragged grouped reductions (all DVE;
        # h0's reductions run before h1's mul so the h0 MLP starts early)
        pm = sb.tile([128, 240], f32, name=f"Pm{h}")
        nc.vector.tensor_mul(fv(pm, 0, [[60, 4], [1, 60]]),
                             fv(ybank, 240 * h, [[60, 4], [1, 60]]),
                             fv(vch[h], 0, [[64, 4], [1, 60]]))
        Pm[h] = pm
        for ws in range(4):
            w = 3 - ws
            o, d = _SLOT_OFF[w], _SLOT_DIM[w]
            nc.vector.reduce_sum(fv(lat_all, 16 * h + 4 * ws, [[1, 4]]),
                                 fv(Pm[h], o, [[60, 4], [1, d]]),
                                 axis=AX.X)

    # ---- MLP in transposed space (shared PSUM banks across halves)
    latT_p = ps.tile([16, 256], f32, tag="latT", bufs=1)
    hT_p = ps.tile([128, 256], f32, tag="hT", bufs=1)
    y4_p = ps.tile([128, 96], f32, tag="y4", bufs=1)
    y_s = sb.tile([128, 96], f32)
    for h in (0, 1):
        nc.tensor.transpose(latT_p[:, 128 * h:128 * h + 128],
                            lat_all[:, 16 * h:16 * h + 16], ident[:, :])
        latT_s = sb2.tile([16, 128], f32r, tag=f"latTs{h}", bufs=1)
        nc.vector.tensor_copy(latT_s[:, :], latT_p[:, 128 * h:128 * h + 128])
        nc.tensor.matmul(hT_p[:, 128 * h:128 * h + 128], lhsT=w1t_v,
                         rhs=latT_s[:, :], start=True, stop=True)
        # relu with b1 folded in as the per-partition ACT bias
        hT_s = sb2.tile([128, 128], f32r, tag=f"hTs{h}", bufs=1)
        nc.scalar.activation(hT_s[:, :], hT_p[:, 128 * h:128 * h + 128],
                             AF.Relu, bias=b1_v, scale=1.0)
        nc.tensor.matmul(y4_p[:, 48 * h:48 * h + 48], lhsT=hT_s[:, :],
                         rhs=w2_v, start=True, stop=True)
        # b2 add doubles as the PSUM->SBUF copy
        nc.vector.tensor_add(fv(y_s, 48 * h, [[1, 48]]),
                             fv(y4_p, 48 * h, [[1, 48]]), b2_v)
        # per-half output DMA: both on the SP queue (idle at that point)
        ya = bass.AP(tensor=y.tensor, offset=48 * h, ap=[[96, 128], [1, 48]])
        nc.sync.dma_start(ya, y_s[:, 48 * h:48 * h + 48])

    return xdma


def _hoist_pre_barrier(nc, inst):
    """Move `inst` (a BassInstruction) into the entry block before the first
    SP-engine instruction (i.e. before the all-engine start barrier)."""
    from concourse import mybir
    ins = inst.ins
    fn = nc.m.functions[0]
    blocks = fn.blocks
    src = None
    for b in blocks:
        for i2 in b.instructions:
            if i2.name == ins.name:
                src = b
                break
        if src is not None:
            break
    assert src is not None, "hoist: dma instruction not found"
    entry = blocks[0]
    src.instructions.remove(ins)
    idx = 0
    for k, i2 in enumerate(entry.instructions):
        if i2.engine == mybir.EngineType.SP:
            idx = k
            break
    entry.instructions.insert(idx, ins)


_NC_CACHE = {}


def _get_nc():
    if "nc" in _NC_CACHE:
        return _NC_CACHE["nc"]
    from contextlib import ExitStack
    import concourse.bacc as bacc
    import concourse.tile as tile
    from concourse import mybir
    f32 = mybir.dt.float32
    nc = bacc.Bacc("TRN2", target_bir_lowering=False, debug=False)
    x = nc.dram_tensor("x", [BLOC, 12], f32, kind="ExternalInput").ap()
    pecst = nc.dram_tensor("pecst", [128, 296], f32, kind="ExternalInput").ap()
    vcst = nc.dram_tensor("vcst", [128, 52], f32, kind="ExternalInput").ap()
    y = nc.dram_tensor("y", [BLOC, 12], f32, kind="ExternalOutput").ap()
    with tile.TileContext(nc) as tc:
        with ExitStack() as ctx:
            xdma = _build_body(ctx, tc, x, pecst, vcst, y)
    _hoist_pre_barrier(nc, xdma)
    nc.compile()
    _NC_CACHE["nc"] = nc
    return nc


def _run(inputs_np, consts, trace=False):
    from concourse.bass_utils import run_bass_kernel_spmd
    nc = _get_nc()
    x = np.ascontiguousarray(np.asarray(inputs_np, np.float32))
    in_maps = []
    for c in range(NCORES):
        m = {"x": np.ascontiguousarray(x[BLOC * c:BLOC * (c + 1)])}
        m.update(consts)
        in_maps.append(m)
    res = run_bass_kernel_spmd(nc, in_maps, core_ids=list(range(NCORES)),
                               trace=trace)
    out = np.concatenate([r["y"] for r in res.results], axis=0)
    return out.astype(np.float32), res


def kernel(inputs, q_params, W1, b1, W2, b2):
    consts = _host_consts(q_params, W1, b1, W2, b2)
    out, _ = _run(inputs, consts, trace=False)
    return out


# revision 17
# speedup vs baseline: 1.1005x; 1.1005x over previous
"""Trainium2 Bass kernel for nn_AutoencoderHybrid (12-qubit QAE hybrid).

Math: the circuit measures Z on wires 0..3 only. The CNOT chain (i -> i+1)
propagates information forward only, so in the Heisenberg picture each
observable Z_w pulled back through the 2-layer circuit is supported on wires
0..w+1 (at most wires 0..4). With the product input state
|psi_b> = kron_j [cos(x_j/2), -i sin(x_j/2)], the diagonal phase factors
D = diag((-i)^popcount) fold into the observable, leaving a REAL quadratic
form on a real product vector. Moreover S_w = Stil_w (x) I_{2^(3-w)} — the
quadform for observable w contracts over only d_w = 2^(w+2) dims:

    latent_w(b) = v_w^T Stil_w v_w,  v_w = kron_{j<=w+1} [cos(x_j/2), sin(x_j/2)]

The prefix vectors A(4) = v_0, B(8) = v_1, D(16) = v_2, R(32) = v_3 all fall
out of the same kron tree. Device computes per batch row the concat vector
vcat = [R|D|B|A] (60 slots, 64-padded) per group, one PE matmul against the
block-diagonal prefix matrix (240 cols vs 512 for the dense form), an
elementwise multiply + ragged grouped reductions, then the tiny decoder MLP
in transposed space (b1 folded into the ACT relu bias, b2 into the final
PSUM->SBUF add).

S_w / MLP weights are tiny (depend only on q_params etc.) and are prepared on
the host; all batch-dim work (B = 8192) runs on 8 NeuronCores, data parallel,
1024 rows per core laid out as b = 8p + c (p partition, c free-dim group).

Scheduling: the x DMA instruction is hoisted into the entry block BEFORE the
all-engine start barrier (saves ~700ns of DMA pipeline latency); constants
arrive as two packed DMAs right behind it on the SP queue; the two output
halves are DMA'd out independently (SP + ACT queues) as soon as each is
written.
"""
import math
import numpy as np

N5 = 5
NLAYERS = 2
LATENT = 4
B = 8192
NCORES = 8
BLOC = B // NCORES  # 1024

# slot layout per group inside vcat (64-wide): [R:0..32|D:32..48|B:48..56|A:56..60]
_SLOT_OFF = {3: 0, 2: 32, 1: 48, 0: 56}
_SLOT_DIM = {3: 32, 2: 16, 1: 8, 0: 4}

# ----------------------------------------------------------------------------
# Host-side constant construction (pure numpy)
# ----------------------------------------------------------------------------


def _rot(phi, theta, omega):
    c, s = np.cos(theta / 2), np.sin(theta / 2)
    ep = np.exp(-0.5j * (phi + omega))
    em = np.exp(-0.5j * (phi - omega))
    return np.array([[ep * c, -np.conj(em) * s], [em * s, np.conj(ep) * c]],
                    dtype=np.complex128)


def _build_S(q_params):
    """(4, 32, 32) real symmetric: latent_w = r^T S_w r (unit-norm slots)."""
    qp = np.asarray(q_params, np.float64)
    dim = 2 ** N5
    eye2 = np.eye(2)

    def kron_at(U, wire):
        M = np.array([[1.0]])
        for j in range(N5):
            M = np.kron(M, U if j == wire else eye2)
        return M

    def cnot_mat(c, t):
        M = np.zeros((dim, dim))
        for z in range(dim):
            bits = [(z >> (N5 - 1 - j)) & 1 for j in range(N5)]
            if bits[c] == 1:
                bits[t] ^= 1
            z2 = 0
            for b in bits:
                z2 = (z2 << 1) | b
            M[z2, z] = 1.0
        return M

    V = np.eye(dim, dtype=np.complex128)
    for l in range(NLAYERS):
        for i in range(N5):
            V = kron_at(_rot(*qp[l, i]), i) @ V
        for i in range(N5 - 1):
            V = cnot_mat(i, i + 1) @ V

    pc = np.array([bin(z).count("1") for z in range(dim)])
    D = np.diag((-1j) ** pc)
    VD = V @ D
    Ss = []
    for w in range(LATENT):
        zdiag = np.array([1.0 if ((z >> (N5 - 1 - w)) & 1) == 0 else -1.0
                          for z in range(dim)])
        O = VD.conj().T @ (zdiag[:, None] * VD)
        Ss.append(np.real(O))
    return np.stack(Ss)


def _host_consts(q_params, W1, b1, W2, b2):
    S = _build_S(q_params)  # (4, 32, 32), unscaled
    # prefix block-diagonal quadform matrix, device slots carry cos/2, sin/2
    # so fold 4^(w+2) per observable
    M1 = np.zeros((64, 60))
    for w in range(4):
        d = _SLOT_DIM[w]
        o = _SLOT_OFF[w]
        tail = 32 // d
        M1[o:o + d, o:o + d] = S[w][::tail, ::tail] * (4.0 ** (w + 2))
    mproj = np.zeros((128, 120), np.float32)
    mproj[0:64, 0:60] = M1
    mproj[64:128, 60:120] = M1

    W1 = np.asarray(W1, np.float64)
    b1 = np.asarray(b1, np.float64)
    W2 = np.asarray(W2, np.float64)
    b2 = np.asarray(b2, np.float64)

    # lat layout [p, 4*ws + g], ws order [w3, w2, w1, w0]
    w1t = np.zeros((16, 128), np.float32)
    for ws in range(4):
        w = 3 - ws
        for g in range(4):
            w1t[4 * ws + g, 32 * g:32 * g + 32] = W1[:, w]
    w2blk = np.zeros((128, 48), np.float32)
    for g in range(4):
        w2blk[32 * g:32 * g + 32, 12 * g:12 * g + 12] = W2.T

    # PE consts: [mproj(0:120) | w2blk(120:168) | w1t(168:296 rows 0:16)]
    pecst = np.zeros((128, 296), np.float32)
    pecst[:, 0:120] = mproj
    pecst[:, 120:168] = w2blk
    pecst[0:16, 168:296] = w1t

    # vector consts: [b2rep(0:48) | b1T(48)]
    vcst = np.zeros((128, 52), np.float32)
    vcst[:, 0:48] = np.tile(b2, 4)[None, :]
    vcst[:, 48] = np.tile(b1, 4)
    return dict(pecst=pecst, vcst=vcst)


# ----------------------------------------------------------------------------
# Device kernel body (Bass/Tile)
# ----------------------------------------------------------------------------


def _build_body(ctx, tc, x, pecst, vcst, y):
    import concourse.bass as bass
    from concourse import mybir
    nc = tc.nc
    f32 = mybir.dt.float32
    f32r = mybir.dt.float32r
    AF = mybir.ActivationFunctionType
    AX = mybir.AxisListType
    ALU = mybir.AluOpType

    def fv(t, col, dims):
        """View of tile t at free-offset col with custom free dims."""
        return bass.AP(tensor=t.tensor, offset=t.offset + col,
                       ap=[list(t.ap[0])] + [list(d) for d in dims])

    consts = ctx.enter_context(tc.tile_pool(name="consts", bufs=1))
    sb = ctx.enter_context(tc.tile_pool(name="sb", bufs=1))
    sb2 = ctx.enter_context(tc.tile_pool(name="sb2", bufs=2))
    ps = ctx.enter_context(tc.tile_pool(name="ps", bufs=2, space="PSUM"))

    # ---- x load: hoisted before the entry barrier post-schedule (SP queue)
    x_s = sb.tile([128, 96], f32)
    xa = bass.AP(tensor=x.tensor, offset=0, ap=[[96, 128], [1, 96]])
    xdma = nc.sync.dma_start(x_s[:, :], xa)

    # ---- packed constants, also SP queue (issues right behind x)
    pe_s = consts.tile([128, 296], f32r)
    nc.sync.dma_start(pe_s[:, :], pecst.bitcast(f32r))
    v_s = consts.tile([128, 52], f32)
    nc.sync.dma_start(v_s[:, :], vcst)
    mproj_v = pe_s[:, 0:120]
    w2_v = pe_s[:, 120:168]
    w1t_v = pe_s[0:16, 168:296]
    b2_v = v_s[:, 0:48]
    b1_v = v_s[:, 48:49]

    bias_c = consts.tile([128, 1], f32)
    nc.vector.memset(bias_c[:, :], math.pi / 2)
    # ident built entirely on Pool (idle at start, no cross-engine hop)
    ones128 = consts.tile([128, 128], f32)
    nc.gpsimd.memset(ones128[:, :], 1.0)
    ident = consts.tile([128, 128], f32)
    nc.gpsimd.affine_select(out=ident[:, :], in_=ones128[:, :],
                            pattern=[[1, 128]],
                            compare_op=ALU.is_equal, fill=0.0,
                            base=0, channel_multiplier=-1)

    # warm the ACT Sin table immediately: a single-dep activation keeps the
    # auto-inserted LoadActFuncSet ahead of any multi-wait semaphore bundle
    warm = consts.tile([128, 1], f32)
    nc.scalar.activation(warm[:, :], bias_c[:, 0:1], AF.Sin,
                         bias=0.0, scale=1.0)

    # ---- quarter angle: c4 = cos(x/4) FIRST (sq/cs_c only need c4), then s4
    # both halves in one op each: sc4[p, 10c + j] = s4, 10c + 5 + j = c4
    sc4 = sb.tile([128, 80], f32)
    xin = fv(x_s, 0, [[12, 8], [1, 5]])
    nc.scalar.activation(fv(sc4, 5, [[10, 8], [1, 5]]), xin, AF.Sin,
                         bias=bias_c[:, 0:1], scale=0.25)
    nc.scalar.activation(fv(sc4, 0, [[10, 8], [1, 5]]), xin, AF.Sin,
                         bias=0.0, scale=0.25)

    # ---- per-half slot assembly + prefix kron tree into vcat
    # cs[p, 10c + j] = cos(x_j/2)/2 = c4^2 - 1/2 ; 10c+5+j = sin(x_j/2)/2
    # vcat per group g (64-aligned): [R(32) | D(16) | B(8) | A(4) | pad]
    vch, csh = [], []
    for h in (0, 1):
        E = nc.vector if h == 0 else nc.gpsimd
        sqt = sb.tile([128, 20], f32, name=f"sq{h}")
        cst = sb.tile([128, 40], f32, name=f"cs{h}")
        # sq and cs_c depend only on c4 (first sin op); cs_s needs s4 too
        E.tensor_mul(fv(sqt, 0, [[5, 4], [1, 5]]),
                     fv(sc4, 40 * h + 5, [[10, 4], [1, 5]]),
                     fv(sc4, 40 * h + 5, [[10, 4], [1, 5]]))
        E.tensor_scalar_sub(fv(cst, 0, [[10, 4], [1, 5]]),
                            fv(sqt, 0, [[5, 4], [1, 5]]), 0.5)
        E.tensor_mul(fv(cst, 5, [[10, 4], [1, 5]]),
                     fv(sc4, 40 * h, [[10, 4], [1, 5]]),
                     fv(sc4, 40 * h + 5, [[10, 4], [1, 5]]))
        vc = sb.tile([128, 256], f32, name=f"vc{h}")
        # A[64g + 56 + 2 z0 + z1] = u0[z0] * u1[z1]
        E.tensor_mul(fv(vc, 56, [[64, 4], [2, 2], [1, 2]]),
                     fv(cst, 1, [[10, 4], [0, 2], [5, 2]]),
                     fv(cst, 0, [[10, 4], [5, 2], [0, 2]]))
        # B[64g + 48 + 2 a + z2] = A[a] * u2[z2]
        E.tensor_mul(fv(vc, 48, [[64, 4], [2, 4], [1, 2]]),
                     fv(vc, 56, [[64, 4], [1, 4], [0, 2]]),
                     fv(cst, 2, [[10, 4], [0, 4], [5, 2]]))
        # D[64g + 32 + 2 b + z3] = B[b] * u3[z3]
        E.tensor_mul(fv(vc, 32, [[64, 4], [2, 8], [1, 2]]),
                     fv(vc, 48, [[64, 4], [1, 8], [0, 2]]),
                     fv(cst, 3, [[10, 4], [0, 8], [5, 2]]))
        # R[64g + 2 d + z4] = D[d] * u4[z4]
        E.tensor_mul(fv(vc, 0, [[64, 4], [2, 16], [1, 2]]),
                     fv(vc, 32, [[64, 4], [1, 16], [0, 2]]),
                     fv(cst, 4, [[10, 4], [0, 16], [5, 2]]))
        vch.append(vc)
        csh.append(cst)

    # ---- per half: PE transposes -> DVE copies -> PE quadform matmuls
    # (PE in-order: keep h0's matmuls ahead of h1's transposes is WRONG --
    # emit per-half T,T,MM,MM so h0's matmuls don't wait on h1's kron)
    ybank = ps.tile([128, 480], f32, tag="Y", bufs=1)
    lat_all = sb.tile([128, 32], f32)
    Pm = {}
    for h in (0, 1):
        tps = []
        for c in (0, 1):
            tp = ps.tile([128, 128], f32, tag="tp", bufs=4)
            nc.tensor.transpose(tp[:, :], vch[h][:, 128 * c:128 * c + 128],
                                ident[:, :])
            tps.append(tp)
        vTs = []
        for c in (0, 1):
            vs = sb2.tile([128, 128], f32r, tag=f"vT{h}{c}", bufs=1)
            nc.vector.tensor_copy(vs[:, :], tps[c][:, :])
            vTs.append(vs)
        for c in (0, 1):
            o = 240 * h + 120 * c
            nc.tensor.matmul(ybank[:, o:o + 120], lhsT=vTs[c][:, :],
                             rhs=mproj_v, start=True, stop=True)
        # Pm = Y * vcat slots, then ragged grouped reductions (all DVE;
        # h0's reductions run before h1's mul so the h0 MLP starts early)
        pm = sb.tile([128, 240], f32, name=f"Pm{h}")
        nc.vector.tensor_mul(fv(pm, 0, [[60, 4], [1, 60]]),
                             fv(ybank, 240 * h, [[60, 4], [1, 60]]),
                             fv(vch[h], 0, [[64, 4], [1, 60]]))
        Pm[h] = pm
        for ws in range(4):
            w = 3 - ws
            o, d = _SLOT_OFF[w], _SLOT_DIM[w]
            nc.vector.reduce_sum(fv(lat_all, 16 * h + 4 * ws, [[1, 4]]),
                                 fv(Pm[h], o, [[60, 4], [1, d]]),
                                 axis=AX.X)

    # ---- MLP in transposed space (shared PSUM banks across halves)
    latT_p = ps.tile([16, 256], f32, tag="latT", bufs=1)
    hT_p = ps.tile([128, 256], f32, tag="hT", bufs=1)
    y4_p = ps.tile([128, 96], f32, tag="y4", bufs=1)
    y_s = sb.tile([128, 96], f32)
    for h in (0, 1):
        nc.tensor.transpose(latT_p[:, 128 * h:128 * h + 128],
                            lat_all[:, 16 * h:16 * h + 16], ident[:, :])
        latT_s = sb2.tile([16, 128], f32r, tag=f"latTs{h}", bufs=1)
        nc.vector.tensor_copy(latT_s[:, :], latT_p[:, 128 * h:128 * h + 128])
        nc.tensor.matmul(hT_p[:, 128 * h:128 * h + 128], lhsT=w1t_v,
                         rhs=latT_s[:, :], start=True, stop=True)
        # relu with b1 folded in as the per-partition ACT bias
        hT_s = sb2.tile([128, 128], f32r, tag=f"hTs{h}", bufs=1)
        nc.scalar.activation(hT_s[:, :], hT_p[:, 128 * h:128 * h + 128],
                             AF.Relu, bias=b1_v, scale=1.0)
        nc.tensor.matmul(y4_p[:, 48 * h:48 * h + 48], lhsT=hT_s[:, :],
                         rhs=w2_v, start=True, stop=True)
        # b2 add doubles as the PSUM->SBUF copy
        nc.vector.tensor_add(fv(y_s, 48 * h, [[1, 48]]),
                             fv(y4_p, 48 * h, [[1, 48]]), b2_v)
        # per-half output DMA: both on the SP queue (idle at that point)
        ya = bass.AP(tensor=y.tensor, offset=48 * h, ap=[[96, 128], [1, 48]])
        nc.sync.dma_start(ya, y_s[:, 48 * h:48 * h + 48])

    return xdma


def _hoist_pre_barrier(nc, inst):
    """Move `inst` (a BassInstruction) into the entry block before the first
    SP-engine instruction (i.e. before the all-engine start barrier)."""
    from concourse import mybir
    ins = inst.ins
    fn = nc.m.functions[0]
    blocks = fn.blocks
    src = None
    for b in blocks:
        for i2 in b.instructions:
            if i2.name == ins.name:
                src = b
                break
        if src is not None:
            break
    assert src is not None, "hoist: dma instruction not found"
    entry = blocks[0]
    src.instructions.remove(ins)
    idx = 0
    for k, i2 in enumerate(entry.instructions):
        if i2.engine == mybir.EngineType.SP:
            idx = k
            break
    entry.instructions.insert(idx, ins)


_NC_CACHE = {}


def _get_nc():
    if "nc" in _NC_CACHE:
        return _NC_CACHE["nc"]
    from contextlib import ExitStack
    import concourse.bacc as bacc
    import concourse.tile as tile
    from concourse import mybir
    f32 = mybir.dt.float32
    nc = bacc.Bacc("TRN2", target_bir_lowering=False, debug=False)
    x = nc.dram_tensor("x", [BLOC, 12], f32, kind="ExternalInput").ap()
    pecst = nc.dram_tensor("pecst", [128, 296], f32, kind="ExternalInput").ap()
    vcst = nc.dram_tensor("vcst", [128, 52], f32, kind="ExternalInput").ap()
    y = nc.dram_tensor("y", [BLOC, 12], f32, kind="ExternalOutput").ap()
    with tile.TileContext(nc) as tc:
        with ExitStack() as ctx:
            xdma = _build_body(ctx, tc, x, pecst, vcst, y)
    _hoist_pre_barrier(nc, xdma)
    nc.compile()
    _NC_CACHE["nc"] = nc
    return nc


def _run(inputs_np, consts, trace=False):
    from concourse.bass_utils import run_bass_kernel_spmd
    nc = _get_nc()
    x = np.ascontiguousarray(np.asarray(inputs_np, np.float32))
    in_maps = []
    for c in range(NCORES):
        m = {"x": np.ascontiguousarray(x[BLOC * c:BLOC * (c + 1)])}
        m.update(consts)
        in_maps.append(m)
    res = run_bass_kernel_spmd(nc, in_maps, core_ids=list(range(NCORES)),
                               trace=trace)
    out = np.concatenate([r["y"] for r in res.results], axis=0)
    return out.astype(np.float32), res


def kernel(inputs, q_params, W1, b1, W2, b2):
    consts = _host_consts(q_params, W1, b1, W2, b2)
    out, _ = _run(inputs, consts, trace=False)
    return out


# revision 18
# speedup vs baseline: 1.3631x; 1.2386x over previous
"""Trainium2 Bass kernel for nn_AutoencoderHybrid (12-qubit QAE hybrid).

Math: the circuit measures Z on wires 0..3 only. The CNOT chain (i -> i+1)
propagates information forward only, so each observable Z_w pulled back
through the 2-layer circuit is supported on wires 0..w+1 (at most 0..4).
With the product input state and the diagonal phase fold, each latent is a
REAL quadratic form over the prefix product vector of dim d_w = 2^(w+2):

    latent_w(b) = v_w^T Stil_w v_w,  v_w = kron_{j<=w+1} [cos(x_j/2), sin(x_j/2)]

The prefix vectors A(4)=v_0, B(8)=v_1, D(16)=v_2, R(32)=v_3 all fall out of
one kron tree, concatenated into vcat (60 slots/group, 64-padded, GROUP-MINOR
layout col = 4*slot + g so every DVE op is packed for fp16 2x mode).

Fully transposed dataflow after the kron (no reductions, no lat tile):
  vcatT   (PE transpose, fp16)
  YT      = mprojT^T @ vcatT        (PE, feature-space quadform halves)
  PmT     = YT * vcatT              (DVE, partition-aligned elementwise)
  hT      = W1X0^T @ PmT0 + W1X1^T @ PmT1   (PE, accumulate; the grouped
            reduction AND the W1 layer folded into one constant matrix)
  hT_s    = relu(hT + b1)           (ACT, bias per-partition)
  y4      = hT_s^T @ w2blk          (PE)
  y       = y4 + b2                 (DVE add doubles as PSUM->SBUF copy)

All constants (mproj/W1X/w2 fp16, b1/b2 f32) are host-prepared; batch work
(B = 8192) runs on 8 NeuronCores data parallel, 1024 rows/core as b = 8p+c.

Scheduling: x DMA hoisted before the entry all-engine barrier; constants in
two packed DMAs behind it on SP; per-half output DMAs issued as ready.
"""
import math
import numpy as np

N5 = 5
NLAYERS = 2
LATENT = 4
B = 8192
NCORES = 8
BLOC = B // NCORES  # 1024

# slot layout per group: [R:0..32 | D:32..48 | B:48..56 | A:56..60 | pad]
_SLOT_OFF = {3: 0, 2: 32, 1: 48, 0: 56}
_SLOT_DIM = {3: 32, 2: 16, 1: 8, 0: 4}

# ----------------------------------------------------------------------------
# Host-side constant construction (pure numpy)
# ----------------------------------------------------------------------------


def _rot(phi, theta, omega):
    c, s = np.cos(theta / 2), np.sin(theta / 2)
    ep = np.exp(-0.5j * (phi + omega))
    em = np.exp(-0.5j * (phi - omega))
    return np.array([[ep * c, -np.conj(em) * s], [em * s, np.conj(ep) * c]],
                    dtype=np.complex128)


def _build_S(q_params):
    """(4, 32, 32) real symmetric: latent_w = r^T S_w r (unit-norm slots)."""
    qp = np.asarray(q_params, np.float64)
    dim = 2 ** N5
    eye2 = np.eye(2)

    def kron_at(U, wire):
        M = np.array([[1.0]])
        for j in range(N5):
            M = np.kron(M, U if j == wire else eye2)
        return M

    def cnot_mat(c, t):
        M = np.zeros((dim, dim))
        for z in range(dim):
            bits = [(z >> (N5 - 1 - j)) & 1 for j in range(N5)]
            if bits[c] == 1:
                bits[t] ^= 1
            z2 = 0
            for b in bits:
                z2 = (z2 << 1) | b
            M[z2, z] = 1.0
        return M

    V = np.eye(dim, dtype=np.complex128)
    for l in range(NLAYERS):
        for i in range(N5):
            V = kron_at(_rot(*qp[l, i]), i) @ V
        for i in range(N5 - 1):
            V = cnot_mat(i, i + 1) @ V

    pc = np.array([bin(z).count("1") for z in range(dim)])
    D = np.diag((-1j) ** pc)
    VD = V @ D
    Ss = []
    for w in range(LATENT):
        zdiag = np.array([1.0 if ((z >> (N5 - 1 - w)) & 1) == 0 else -1.0
                          for z in range(dim)])
        O = VD.conj().T @ (zdiag[:, None] * VD)
        Ss.append(np.real(O))
    return np.stack(Ss)


def _host_consts(q_params, W1, b1, W2, b2):
    S = _build_S(q_params)
    W1 = np.asarray(W1, np.float64)
    b1 = np.asarray(b1, np.float64)
    W2 = np.asarray(W2, np.float64)
    b2 = np.asarray(b2, np.float64)

    def stil(w):
        d = _SLOT_DIM[w]
        tail = 32 // d
        # device slots carry cos/2, sin/2 -> fold 4^(w+2)
        return S[w][::tail, ::tail] * (4.0 ** (w + 2))

    # mproj chunk0: rows (4i+g) i<32 (R slots) -> w3 outs (4i'+g)
    mp0 = np.zeros((128, 128))
    St3 = stil(3)
    for g in range(4):
        mp0[g::4, g::4] = St3.T
    # chunk1: rows (4s+g), s = slot-32 ([D|B|A|pad]) -> oslots 32..60
    mp1 = np.zeros((128, 112))
    for w, so in ((2, 0), (1, 16), (0, 24)):
        Sw = stil(w)
        d = _SLOT_DIM[w]
        for g in range(4):
            mp1[4 * so + g:4 * (so + d) + g:4,
                4 * so + g:4 * (so + d) + g:4] = Sw.T

    # W1X chunks fold the grouped reduction + W1: rows k -> (slot, g),
    # cols (32g + a)
    def w1x_chunk(c, rows):
        M = np.zeros((rows, 128))
        for k in range(rows):
            slot = (128 * c + k) >> 2
            g = k & 3
            if slot < 32:
                w = 3
            elif slot < 48:
                w = 2
            elif slot < 56:
                w = 1
            elif slot < 60:
                w = 0
            else:
                continue
            M[k, 32 * g:32 * g + 32] = W1[:, w]
        return M
    W1X0 = w1x_chunk(0, 128)
    W1X1 = w1x_chunk(1, 128)  # rows 112.. are zero (pad slots)

    w2blk = np.zeros((128, 48))
    for g in range(4):
        w2blk[32 * g:32 * g + 32, 12 * g:12 * g + 12] = W2.T

    # fp16 packed consts: [mp0 | mp1 | W1X0 | W1X1 | w2]
    hcst = np.zeros((128, 544), np.float16)
    hcst[:, 0:128] = mp0
    hcst[:, 128:240] = mp1
    hcst[:, 240:368] = W1X0
    hcst[:, 368:496] = W1X1
    hcst[:, 496:544] = w2blk

    # f32 consts: [b2rep | b1T]
    fcst = np.zeros((128, 52), np.float32)
    fcst[:, 0:48] = np.tile(b2, 4)[None, :]
    fcst[:, 48] = np.tile(b1, 4)
    return dict(hcst=hcst, fcst=fcst)


# ----------------------------------------------------------------------------
# Device kernel body (Bass/Tile)
# ----------------------------------------------------------------------------


def _build_body(ctx, tc, x, hcst, fcst, y):
    import concourse.bass as bass
    from concourse import mybir
    nc = tc.nc
    f32 = mybir.dt.float32
    f16 = mybir.dt.float16
    AF = mybir.ActivationFunctionType
    ALU = mybir.AluOpType

    def fv(t, col, dims):
        """View of tile t at free-offset col with custom free dims."""
        return bass.AP(tensor=t.tensor, offset=t.offset + col,
                       ap=[list(t.ap[0])] + [list(d) for d in dims])

    consts = ctx.enter_context(tc.tile_pool(name="consts", bufs=1))
    sb = ctx.enter_context(tc.tile_pool(name="sb", bufs=1))
    sb2 = ctx.enter_context(tc.tile_pool(name="sb2", bufs=2))
    ps = ctx.enter_context(tc.tile_pool(name="ps", bufs=2, space="PSUM"))

    # ---- x load: hoisted before the entry barrier post-schedule (SP queue)
    x_s = sb.tile([128, 96], f32)
    xa = bass.AP(tensor=x.tensor, offset=0, ap=[[96, 128], [1, 96]])
    xdma = nc.sync.dma_start(x_s[:, :], xa)

    # ---- packed constants, also SP queue (issue right behind x)
    h_s = consts.tile([128, 544], f16)
    nc.sync.dma_start(h_s[:, :], hcst)
    f_s = consts.tile([128, 52], f32)
    nc.sync.dma_start(f_s[:, :], fcst)
    mp0_v = h_s[:, 0:128]
    mp1_v = h_s[:, 128:240]
    w1x0_v = h_s[:, 240:368]
    w1x1_v = h_s[0:112, 368:496]
    w2_v = h_s[:, 496:544]
    b2_v = f_s[:, 0:48]
    b1_v = f_s[:, 48:49]

    bias_c = consts.tile([128, 1], f32)
    nc.vector.memset(bias_c[:, :], math.pi / 2)
    # fp16 identity for the transposes, built on Pool (idle at start)
    ones16 = consts.tile([128, 128], f16)
    nc.gpsimd.memset(ones16[:, :], 1.0)
    ident = consts.tile([128, 128], f16)
    nc.gpsimd.affine_select(out=ident[:, :], in_=ones16[:, :],
                            pattern=[[1, 128]],
                            compare_op=ALU.is_equal, fill=0.0,
                            base=0, channel_multiplier=-1)

    # warm the ACT Sin table immediately (single-dep activation keeps the
    # auto-inserted LoadActFuncSet ahead of any multi-wait semaphore bundle)
    warm = consts.tile([128, 1], f32)
    nc.scalar.activation(warm[:, :], bias_c[:, 0:1], AF.Sin,
                         bias=0.0, scale=1.0)

    # vcat tiles + zero their pad slots early (pads flow into the transposes)
    vch = []
    for h in (0, 1):
        vc = sb.tile([128, 256], f16, name=f"vc{h}")
        E = nc.vector if h == 0 else nc.gpsimd
        E.memset(vc[:, 240:256], 0.0)
        vch.append(vc)

    # ---- quarter angle: c4 = cos(x/4) FIRST (sq/cs_c only need c4), then s4
    # sc4 layout (group-minor, per half at 40h): col = 40h + 20*t4 + 4j + g
    sc4 = sb.tile([128, 80], f16)
    for t4, bias in ((1, bias_c[:, 0:1]), (0, 0.0)):
        nc.scalar.activation(fv(sc4, 20 * t4, [[40, 2], [4, 5], [1, 4]]),
                             fv(x_s, 0, [[48, 2], [1, 5], [12, 4]]),
                             AF.Sin, bias=bias, scale=0.25)

    # ---- per-half slot assembly + prefix kron tree into vcat (all packed)
    # cs layout: col = 20*t + 4j + g  (t=0: cos(x/2)/2, t=1: sin(x/2)/2)
    for h in (0, 1):
        E = nc.vector if h == 0 else nc.gpsimd
        sqt = sb.tile([128, 20], f16, name=f"sq{h}")
        cst = sb.tile([128, 40], f16, name=f"cs{h}")
        ctmp = sb.tile([128, 16], f16, name=f"ct{h}")
        vc = vch[h]
        c4 = lambda j: fv(sc4, 40 * h + 20 + 4 * j, [[1, 4]])
        # sq = c4^2 ; cs_c = sq - 1/2 ; cs_s = s4*c4
        E.tensor_mul(fv(sqt, 0, [[4, 5], [1, 4]]),
                     fv(sc4, 40 * h + 20, [[4, 5], [1, 4]]),
                     fv(sc4, 40 * h + 20, [[4, 5], [1, 4]]))
        E.tensor_scalar_sub(fv(cst, 0, [[4, 5], [1, 4]]),
                            fv(sqt, 0, [[4, 5], [1, 4]]), 0.5)
        E.tensor_mul(fv(cst, 20, [[4, 5], [1, 4]]),
                     fv(sc4, 40 * h, [[4, 5], [1, 4]]),
                     fv(sc4, 40 * h + 20, [[4, 5], [1, 4]]))
        # u_j[t, g] at cst col 20t + 4j + g
        # A[a=(z0 z1)] -> vcat slots 56..60 (cols 224..240)
        E.tensor_mul(fv(vc, 224, [[8, 2], [4, 2], [1, 4]]),
                     fv(cst, 4, [[0, 2], [20, 2], [1, 4]]),
                     fv(cst, 0, [[20, 2], [0, 2], [1, 4]]))
        # C[(z3 z4)] -> ctmp
        E.tensor_mul(fv(ctmp, 0, [[8, 2], [4, 2], [1, 4]]),
                     fv(cst, 16, [[0, 2], [20, 2], [1, 4]]),
                     fv(cst, 12, [[20, 2], [0, 2], [1, 4]]))
        # B[(a z2)] -> slots 48..56 (cols 192..224)
        E.tensor_mul(fv(vc, 192, [[8, 4], [4, 2], [1, 4]]),
                     fv(vc, 224, [[4, 4], [0, 2], [1, 4]]),
                     fv(cst, 8, [[0, 4], [20, 2], [1, 4]]))
        # R[(b c2)] -> slots 0..32 (cols 0..128)   (chunk0 = pure R)
        E.tensor_mul(fv(vc, 0, [[16, 8], [4, 4], [1, 4]]),
                     fv(vc, 192, [[4, 8], [0, 4], [1, 4]]),
                     fv(ctmp, 0, [[0, 8], [4, 4], [1, 4]]))
        # D[(b z3)] -> slots 32..48 (cols 128..192)  (feeds only chunk1)
        E.tensor_mul(fv(vc, 128, [[8, 8], [4, 2], [1, 4]]),
                     fv(vc, 192, [[4, 8], [0, 2], [1, 4]]),
                     fv(cst, 12, [[0, 8], [20, 2], [1, 4]]))

    # ---- per half: transposes -> copies -> feature-space pipeline
    y4_p = ps.tile([128, 96], f32, tag="y4", bufs=1)
    y_s = sb.tile([128, 96], f32)
    for h in (0, 1):
        tps = []
        for c in (0, 1):
            tp = ps.tile([128, 128], f16, tag="tp", bufs=3)
            nc.tensor.transpose(tp[:, :], vch[h][:, 128 * c:128 * c + 128],
                                ident[:, :])
            tps.append(tp)
        vT = []
        for c in (0, 1):
            vs = sb2.tile([128, 128], f16, tag=f"vT{h}{c}", bufs=1)
            if c == 0:
                nc.vector.tensor_copy(vs[:, :], tps[c][:, :])
            else:
                nc.scalar.copy(vs[:, :], tps[c][:, :])
            vT.append(vs)
        # YT = mproj^T @ vcatT (per-half PSUM bank holds both chunks)
        ytb = ps.tile([128, 256], f32, tag=f"YT{h}", bufs=1)
        nc.tensor.matmul(ytb[:, 0:128], lhsT=mp0_v, rhs=vT[0][:, :],
                         start=True, stop=True)
        nc.tensor.matmul(ytb[0:112, 128:256], lhsT=mp1_v, rhs=vT[1][:, :],
                         start=True, stop=True)
        # PmT = YT * vcatT  (partition-aligned elementwise, DVE)
        pm0 = sb2.tile([128, 128], f16, tag=f"Pm{h}0", bufs=1)
        nc.vector.tensor_mul(pm0[:, :], ytb[:, 0:128], vT[0][:, :])
        pm1 = sb2.tile([112, 128], f16, tag=f"Pm{h}1", bufs=1)
        nc.vector.tensor_mul(pm1[:, :], ytb[0:112, 128:256],
                             vT[1][0:112, :])
        # hT = W1X0^T @ PmT0 + W1X1^T @ PmT1 (reduction + W1 in one step)
        hT_p = ps.tile([128, 128], f32, tag=f"hT{h}", bufs=1)
        nc.tensor.matmul(hT_p[:, :], lhsT=w1x0_v, rhs=pm0[:, :],
                         start=True, stop=False)
        nc.tensor.matmul(hT_p[:, :], lhsT=w1x1_v, rhs=pm1[:, :],
                         start=False, stop=True)
        # relu with b1 folded in as the per-partition ACT bias
        hT_s = sb2.tile([128, 128], f16, tag=f"hTs{h}", bufs=1)
        nc.scalar.activation(hT_s[:, :], hT_p[:, :], AF.Relu,
                             bias=b1_v, scale=1.0)
        nc.tensor.matmul(y4_p[:, 48 * h:48 * h + 48], lhsT=hT_s[:, :],
                         rhs=w2_v, start=True, stop=True)
        # b2 add doubles as the PSUM->SBUF copy
        nc.vector.tensor_add(fv(y_s, 48 * h, [[1, 48]]),
                             fv(y4_p, 48 * h, [[1, 48]]), b2_v)
        ya = bass.AP(tensor=y.tensor, offset=48 * h, ap=[[96, 128], [1, 48]])
        nc.sync.dma_start(ya, y_s[:, 48 * h:48 * h + 48])

    return xdma


def _hoist_pre_barrier(nc, inst):
    """Move `inst` (a BassInstruction) into the entry block before the first
    SP-engine instruction (i.e. before the all-engine start barrier)."""
    from concourse import mybir
    ins = inst.ins
    fn = nc.m.functions[0]
    blocks = fn.blocks
    src = None
    for b in blocks:
        for i2 in b.instructions:
            if i2.name == ins.name:
                src = b
                break
        if src is not None:
            break
    assert src is not None, "hoist: dma instruction not found"
    entry = blocks[0]
    src.instructions.remove(ins)
    idx = 0
    for k, i2 in enumerate(entry.instructions):
        if i2.engine == mybir.EngineType.SP:
            idx = k
            break
    entry.instructions.insert(idx, ins)


_NC_CACHE = {}


def _get_nc():
    if "nc" in _NC_CACHE:
        return _NC_CACHE["nc"]
    from contextlib import ExitStack
    import concourse.bacc as bacc
    import concourse.tile as tile
    from concourse import mybir
    f32 = mybir.dt.float32
    f16 = mybir.dt.float16
    nc = bacc.Bacc("TRN2", target_bir_lowering=False, debug=False)
    x = nc.dram_tensor("x", [BLOC, 12], f32, kind="ExternalInput").ap()
    hcst = nc.dram_tensor("hcst", [128, 544], f16, kind="ExternalInput").ap()
    fcst = nc.dram_tensor("fcst", [128, 52], f32, kind="ExternalInput").ap()
    y = nc.dram_tensor("y", [BLOC, 12], f32, kind="ExternalOutput").ap()
    with tile.TileContext(nc) as tc:
        with ExitStack() as ctx:
            xdma = _build_body(ctx, tc, x, hcst, fcst, y)
    _hoist_pre_barrier(nc, xdma)
    nc.compile()
    _NC_CACHE["nc"] = nc
    return nc


def _run(inputs_np, consts, trace=False):
    from concourse.bass_utils import run_bass_kernel_spmd
    nc = _get_nc()
    x = np.ascontiguousarray(np.asarray(inputs_np, np.float32))
    in_maps = []
    for c in range(NCORES):
        m = {"x": np.ascontiguousarray(x[BLOC * c:BLOC * (c + 1)])}
        m.update(consts)
        in_maps.append(m)
    res = run_bass_kernel_spmd(nc, in_maps, core_ids=list(range(NCORES)),
                               trace=trace)
    out = np.concatenate([r["y"] for r in res.results], axis=0)
    return out.astype(np.float32), res


def kernel(inputs, q_params, W1, b1, W2, b2):
    consts = _host_consts(q_params, W1, b1, W2, b2)
    out, _ = _run(inputs, consts, trace=False)
    return out
